# revision 99
# baseline (speedup 1.0000x reference)
"""Multi-head causal attention (B=4, S=2048, D=1024, H=16) on 8 NeuronCores.

Sharding: batch x head-group. Core c handles batch b = c//2 and head group
g = c%2 (8 heads of 64 dims each). Wq/Wk/Wv are column-split per head group
(Megatron column-parallel), Wo is row-split; each core returns a partial
output [S, D] (bf16) which the host sums over the two head-group cores.

Device kernel (identical SPMD program on all 8 cores):
  1. QT/KT = (X @ Wg)^T computed in transposed layout (dk on
     partitions), then split into fp8e4 hi/lo pairs (x = hi + lo, each
     fp8) so score matmuls run in fp8 DoubleRow perf mode at 2x PE
     throughput with ~bf16 accuracy:
       kt8[h]: [128, S] fp8, partitions = [K_hi(0:64); K_lo(64:128)].
       qt8[h]: [128, 2, S] fp8, slot0 = Q_hi in both partition halves,
               slot1 = Q_lo in both halves (duplicated).
     Placements are two-stage (cols 0:1024 after js1, rest after js3):
     pr0 units only read the first half, so attention starts right
     after K0/Q0's first two column slabs. V is computed in natural
     layout [sk, dv] bf16 with a ones-column per head (softmax
     denominator trick). x tensors load as quarter-tensor DMAs in
     first-use order (each DMA costs a serial ~625ns HWDGE slot, so
     few/large transfers keep the startup queue short).
  2. Per head: scores tile [sk=128, sq<=1024] = one DoubleRow matmul
     per 512-wide PSUM bank piece: lhsT = kt8 slice broadcast over the
     slot dim (stride 0), rhs = qt8 slice -> (K_hi+K_lo)^T(Q_hi+Q_lo)
     exactly. exp on ScalarE (1/sqrt(dk) folded into the activation
     scale), causal diag masking via 0/1 mask multiply on GpSimd.
     AV runs TRANSPOSED: per 128-wide query piece i,
     psC2[q, i, 0:65] += at[:, 128i:128(i+1)]^T @ [V | ones], so the
     matmul output free size is 65 instead of the query width — half
     the AV PE cost of the direct-C^T orientation. The 8 piece windows
     live at 512B strides inside one 2-bank psum tile; since a
     matmul's start=True wipes its ENTIRE psum bank, the tile is
     zeroed once on DVE and all AV matmuls accumulate (start=False).
     The per-partition denominator makes normalize one [128,8,1]
     reciprocal + a broadcast multiply. The pair's two heads
     interleave normalized context into one [q, i, (head, d)] bf16
     stage; a single XBAR DMA transpose per (pr, pair) then lands the
     exact ct layout [head*64+d, i, q] — no PE transposes, no psum.
  3. Schedule: the attention span is exp(ScalarE)-bound, so PE
     projection work is dribbled INTO it as fillers. Units are grouped
     mixing pr0 pairs with earlier pairs' (independent) pr1 units so
     every filler slab sits inside an exp-rich span: head-pair m's K/Q
     projections fill group m-1. The last group runs pr1(6,7) with the
     pr0 output projection dribbled in; the pr1 output projection
     borrows the then-dead psL/psC2 psum tiles for an 8-slot drain
     pipeline and stores ride the Pool SWDGE queue (skips the serial
     HWDGE). The host sums the two head-group partials.
"""
import json

import numpy as np
import ml_dtypes

BF16 = ml_dtypes.bfloat16
F8 = ml_dtypes.float8_e4m3

B, S, D = 4, 2048, 1024
H = 16
DK = 64          # per-head dim
HPG = 8          # heads per group
GW = HPG * DK    # group width = 512
N_CORES = 8

_nc_cache = {}
DEBUG_DUMPS = False
TUNE = {"apool": 4, "xpool": 4, "xkpool": 4, "every": 6, "every5": 2,
        "pl": 2, "pv": 2, "opool": 5, "vblk": 1, "spool": 2, "cnpool": 2,
        "overlap": 1}


def _apply_compat_patches():
    """This container's walrus rejects instructions carrying more than one
    sem-wait ("Too many sync wait commands"). Split excess waits onto NoOps
    on the same engine, patched into every compile path."""
    import concourse.bass_utils as bass_utils

    if getattr(bass_utils, "_wait_split_patched", False):
        return
    _orig = bass_utils.compile_bir_kernel
    seq = [0]

    def split_bir_waits(bir, limit=1):
        for fn in bir.get("functions", []):
            for bb in fn.get("blocks", []):
                out, changed = [], False
                for ins in bb.get("instructions", []):
                    si = ins.get("sync_info")
                    ow = (si or {}).get("on_wait") or []
                    if len(ow) > limit:
                        changed = True
                        extra, keep = ow[:-limit], ow[-limit:]
                        for i in range(0, len(extra), limit):
                            seq[0] += 1
                            out.append({
                                "debug": ins.get("debug", 0),
                                "engine": ins["engine"],
                                "ins": [], "outs": [],
                                "name": f"WSPLIT-{seq[0]}",
                                "opcode": "NoOp",
                                "sync_info": {"on_update": [],
                                              "on_wait": extra[i:i + limit]},
                            })
                        si["on_wait"] = keep
                    out.append(ins)
                if changed:
                    bb["instructions"] = out
        return bir

    def _patched(bir_json, tmpdir, neff_name="file.neff", **kw):
        bir = split_bir_waits(json.loads(bir_json))
        return _orig(json.dumps(bir).encode(), tmpdir, neff_name, **kw)

    bass_utils.compile_bir_kernel = _patched
    bass_utils._wait_split_patched = True
    try:
        import concourse.bass2jax as bass2jax
        bass2jax.compile_bir_kernel = _patched
    except Exception:
        pass


def build_attention_nc():
    """Build the SPMD Bass program (one NeuronCore's view).

    Emission order interleaves Q-projection with pr=0 attention per head
    pair, and slots the pr=0 output projection into the middle of pr=1
    attention, so ScalarE (exp) and TensorE stay busy together.
    """
    import concourse.bass as bass
    import concourse.mybir as mybir
    import concourse.tile as tile

    fp32 = mybir.dt.float32
    bf16 = mybir.dt.bfloat16
    f8 = mybir.dt.float8e4
    Exp = mybir.ActivationFunctionType.Exp
    DR = mybir.MatmulPerfMode.DoubleRow

    nc = bass.Bass("TRN2", target_bir_lowering=False, debug=False,
                   num_devices=N_CORES)

    xqT = nc.dram_tensor("xqT", [D, S], bf16, kind="ExternalInput")
    xkT = nc.dram_tensor("xkT", [D, S], bf16, kind="ExternalInput")
    xvT = nc.dram_tensor("xvT", [D, S], bf16, kind="ExternalInput")
    wq = nc.dram_tensor("wq", [D, GW], bf16, kind="ExternalInput")
    wk = nc.dram_tensor("wk", [D, GW], bf16, kind="ExternalInput")
    wv = nc.dram_tensor("wv", [D, GW], bf16, kind="ExternalInput")
    wo = nc.dram_tensor("wo", [GW, D], bf16, kind="ExternalInput")
    masks = nc.dram_tensor("masks", [128, 128], bf16, kind="ExternalInput")
    out = nc.dram_tensor("out", [S, D], bf16, kind="ExternalOutput")
    dbg = {}
    if DEBUG_DUMPS:
        dbg["at00"] = nc.dram_tensor("dbg_at00", [128, 1024], bf16,
                                     kind="ExternalOutput")
        dbg["big00"] = nc.dram_tensor("dbg_big00", [128, 8, 128], fp32,
                                      kind="ExternalOutput")
        dbg["cn0"] = nc.dram_tensor("dbg_cn0", [128, 8, 2, DK], bf16,
                                    kind="ExternalOutput")
        dbg["ct0"] = nc.dram_tensor("dbg_ct0", [128, S], bf16,
                                    kind="ExternalOutput")
        dbg["kt0"] = nc.dram_tensor("dbg_kt0", [128, S], f8,
                                    kind="ExternalOutput")
        dbg["qt0"] = nc.dram_tensor("dbg_qt0", [128, 2, S], f8,
                                    kind="ExternalOutput")
        dbg["vt0"] = nc.dram_tensor("dbg_vt0", [128, HPG * (DK + 1)], bf16,
                                    kind="ExternalOutput")

    KC = D // 128
    SQT = S // 512
    SKC = S // 128

    with tile.TileContext(nc) as tc:
        with tc.tile_pool(name="wpool", bufs=1) as wpool, \
             tc.tile_pool(name="xpool", bufs=TUNE["xpool"]) as xpool, \
             tc.tile_pool(name="xkpool", bufs=TUNE["xkpool"]) as xkpool, \
             tc.tile_pool(name="persist", bufs=1) as persist, \
             tc.tile_pool(name="apool", bufs=TUNE["apool"]) as apool, \
             tc.tile_pool(name="rcpool", bufs=TUNE.get("rcpool", 2)) as rcpool, \
             tc.tile_pool(name="cnpool", bufs=TUNE.get("cnpool", 2)) as cnpool, \
             tc.tile_pool(name="wvk", bufs=2) as wvk, \
             tc.tile_pool(name="spool", bufs=TUNE["spool"]) as spool, \
             tc.tile_pool(name="opool", bufs=TUNE["opool"]) as opool, \
             tc.tile_pool(name="pl", bufs=TUNE["pl"], space="PSUM") as pl, \
             tc.tile_pool(name="pv", bufs=TUNE["pv"], space="PSUM") as pv, \
             tc.tile_pool(name="pc2", bufs=1, space="PSUM") as pc2:

            # wv/wk share one rotating buffer (wk load hides behind V
            # compute); wq/wo persist (used through attention / at the end).
            wv_sb = wvk.tile([128, KC, GW], bf16, tag="wvk", name="wv_sb")
            wq_sb = wpool.tile([128, KC, GW], bf16, tag="wq")
            wo_sb = wpool.tile([128, GW // 128, D], bf16, tag="wo")
            mask_sb = wpool.tile([128, 128], bf16, tag="masks")
            # wv chunk loads ride the Act queue (idle at startup) so the SP
            # queue can stream x-chunks immediately.
            nc.scalar.dma_start(wv_sb[:],
                                wv.ap().rearrange("(kc p) m -> p kc m", p=128))

            # fp8 hi/lo Q and K, per head
            qt8 = [persist.tile([128, 2, S], f8, tag=f"qt{h}", name=f"qt{h}")
                   for h in range(HPG)]
            kt8 = [persist.tile([128, S], f8, tag=f"kt{h}", name=f"kt{h}")
                   for h in range(HPG)]
            vt = [persist.tile([128, HPG * (DK + 1)], bf16, tag=f"vt{j}",
                               name=f"vt{j}") for j in range(SKC)]
            ct = [persist.tile([128, S], bf16, tag=f"ct{m}", name=f"ct{m}")
                  for m in range(4)]

            class XHalves:
                """x halves loaded as two [128, 4kc, S/2] quarter-tensor
                DMAs: every DMA costs a serial ~625ns HWDGE slot regardless
                of size, so coarse transfers keep the startup HWDGE queue
                short. `load(half)` order = first-use order."""

                def __init__(self, xT, pool):
                    self.xT, self.pool = xT, pool
                    self.h = [None, None]

                def load(self, half, eng=None):
                    eng = eng or nc.sync
                    cw = slice(half * (S // 2), (half + 1) * (S // 2))
                    xr = self.xT.ap().rearrange("(q k p) s -> q p k s",
                                                q=2, p=128)
                    quarters = []
                    for qq in range(2):
                        xc = self.pool.tile([128, KC // 2, S // 2], bf16,
                                            tag="xch", name="xch")
                        eng.dma_start(xc[:], xr[qq, :, :, cw])
                        quarters.append(xc)
                    self.h[half] = quarters

                def win(self, kc, c0, c1):
                    """[c0, c1) must lie within one S/2 half."""
                    half, base = divmod(c0, S // 2)
                    return self.h[half][kc // 4][:, kc % 4,
                                                 base:base + (c1 - c0)]

            # ---- V projection (natural layout + ones column) -------------
            # j-blocks with kc as the middle loop: the first block's kc=0
            # matmuls only need xch[0], so PE starts ~1.5us after the first
            # x-chunk DMA instead of waiting for all eight.
            xv = XHalves(xvT, xpool)
            xk = XHalves(xkT, xkpool)
            xq = XHalves(xqT, xpool)
            # weight loads ride the Act queue (idle until the first exp);
            # x halves stream on the SP queue in first-use order. xq shares
            # xpool with xv: its chunks take over xv's buffers as the
            # V-projection consumes them.
            wk_sb = wvk.tile([128, KC, GW], bf16, tag="wvk", name="wk_sb")
            nc.scalar.dma_start(wk_sb[:],
                                wk.ap().rearrange("(kc p) m -> p kc m", p=128))
            nc.scalar.dma_start(wq_sb[:],
                                wq.ap().rearrange("(kc p) m -> p kc m", p=128))
            nc.scalar.dma_start(mask_sb[:], masks.ap())
            xv.load(0)
            xk.load(0)
            xv.load(1)
            xk.load(1)
            xq.load(0)
            xq.load(1)
            VB = TUNE["vblk"]

            def vproj_block(jb):
                pss = [pv.tile([128, 512], fp32, tag="pv", name="psv")
                       for _ in range(VB)]
                for kc in range(KC):
                    for i in range(VB):
                        j = jb + i
                        nc.tensor.matmul(
                            pss[i][:], xv.win(kc, 128 * j, 128 * (j + 1)),
                            wv_sb[:, kc, :],
                            start=(kc == 0), stop=(kc == KC - 1))
                for i in range(VB):
                    j = jb + i
                    vt_v = vt[j][:].rearrange("p (h c) -> p h c", c=DK + 1)
                    nc.vector.tensor_copy(
                        vt_v[:, :, 0:DK],
                        pss[i][:].rearrange("p (h c) -> p h c", c=DK))
                    nc.vector.memset(vt_v[:, :, DK:DK + 1], 1.0)



            # ---- K projection -> fp8 hi/lo transposed layout -------------
            # psum chunk [128, 512]: rows 0:64 = head 2m dk, 64:128 = 2m+1.
            # DVE drains psum to a bf16 stage (frees the bank fast), then
            # GpSimd (SBUF-only engine) casts hi and subtracts lo; the
            # opposite partition half is filled by an SBUF->SBUF DMA.
            def hilo_chunk(st, js, ps, on_act=True):
                """fp8 hi/lo split of a [128, 512] psum chunk into the
                js window of a full-S stage tile [128, 2(hi,lo), S].

                hi-cast on ScalarE during the pre-attention phases (ScalarE
                is idle until exp starts); on DVE for the chunks dribbled
                into attention, where ScalarE is saturated by exp."""
                w = slice(512 * js, 512 * (js + 1))
                if on_act:
                    nc.scalar.copy(st[:, 0, w], ps[:])
                else:
                    nc.vector.tensor_copy(st[:, 0, w], ps[:])
                nc.vector.tensor_sub(st[:, 1, w], ps[:], st[:, 0, w])

            # Placement DMAs ride the Pool queue (SWDGE): Pool is idle
            # during production, the dispatch bypasses the serial HWDGE,
            # and they never block the SP queue's x-chunk loads.
            def k_place(m, st, cw, eng=None):
                """4 placement DMAs: stage -> kt8[2m], kt8[2m+1] in block
                [hi(0:64); lo(64:128)] partition layout, columns cw."""
                eng = eng or nc.gpsimd
                he, ho = kt8[2 * m], kt8[2 * m + 1]
                eng.dma_start(he[0:64, cw], st[0:64, 0, cw])
                eng.dma_start(he[64:128, cw], st[0:64, 1, cw])
                eng.dma_start(ho[0:64, cw], st[64:128, 0, cw])
                eng.dma_start(ho[64:128, cw], st[64:128, 1, cw])

            def q_place(m, st, cw, eng=None):
                """4 dup DMAs: stage -> qt8[2m], qt8[2m+1]; slot0=hi,
                slot1=lo, duplicated across both partition halves."""
                eng = eng or nc.gpsimd
                he, ho = qt8[2 * m], qt8[2 * m + 1]
                eng.dma_start(he[0:64, :, cw], st[0:64, :, cw])
                eng.dma_start(he[64:128, :, cw], st[0:64, :, cw])
                eng.dma_start(ho[0:64, :, cw], st[64:128, :, cw])
                eng.dma_start(ho[64:128, :, cw], st[64:128, :, cw])

            kstages, qstages, pair_cn = {}, {}, {}
            FULL, H0, H1 = slice(0, S), slice(0, S // 2), slice(S // 2, S)

            def kproj_js(m, js, pool=None):
                if m not in kstages:
                    kstages[m] = spool.tile([128, 2, S], f8, tag="st",
                                            name="kst")
                ps = (pool or pv).tile([128, 512], fp32,
                                       tag="pv" if pool is None else "pc2",
                                       name="psk")
                for kc in range(KC):
                    nc.tensor.matmul(
                        ps[:], wk_sb[:, kc, 128 * m:128 * (m + 1)],
                        xk.win(kc, 512 * js, 512 * (js + 1)),
                        start=(kc == 0), stop=(kc == KC - 1))
                hilo_chunk(kstages[m], js, ps, on_act=(m == 0 and js < 2))
                # Two-stage placement (cols 0:1024 after js1, rest after
                # js3): pr0 units only read kt[:, 0:1024], so each pair's
                # pr0 deadline needs just the H0 half. m0's H0 goes on the
                # idle-at-prologue Pool SWDGE queue; later placements ride
                # SP so they never block Pool's mask multiplies.
                if js == 1:
                    k_place(m, kstages[m], H0,
                            eng=nc.gpsimd if m == 0 else nc.sync)
                elif js == SQT - 1:
                    k_place(m, kstages.pop(m), H1, eng=nc.sync)

            def qproj_js(m, js, pool=None):
                if m not in qstages:
                    qstages[m] = spool.tile([128, 2, S], f8, tag="st",
                                            name="qst")
                ps = (pool or pv).tile([128, 512], fp32,
                                       tag="pv" if pool is None else "pc2",
                                       name="psq")
                for kc in range(KC):
                    nc.tensor.matmul(
                        ps[:], wq_sb[:, kc, 128 * m:128 * (m + 1)],
                        xq.win(kc, 512 * js, 512 * (js + 1)),
                        start=(kc == 0), stop=(kc == KC - 1))
                hilo_chunk(qstages[m], js, ps, on_act=(m == 0 and js < 2))
                if js == 1:
                    q_place(m, qstages[m], H0,
                            eng=nc.gpsimd if m == 0 else nc.sync)
                elif js == SQT - 1:
                    q_place(m, qstages.pop(m), H1, eng=nc.sync)

            def attn_steps(pr, h):
                """Software-pipelined attention unit: scores(jk+1) is
                emitted BEFORE AV(jk), so the in-order PE queue always has
                the next DoubleRow score matmul ready while ScalarE computes
                exp(jk) — the exp latency never exposes on PE.

                AV runs transposed: per 128-wide query piece i,
                psC2[q, i, d] += sum_sk at[sk, 128i+q] * V[sk, d] — the
                matmul output free size is 65 (64 dv + ones-denominator)
                instead of the query width, halving AV's PE cost. The
                denominator lands per-PARTITION, so normalize is one tiny
                reciprocal + a broadcast multiply (no DMA broadcast), and
                8 cheap PE transposes restore the [dv, sq] ct layout the
                output projection needs. The transpose outputs reuse the
                psC2 psum banks via a second (bf16) tile allocation."""
                kt_h = kt8[h]
                qt_h = qt8[h]
                nK = 8 * (pr + 1)
                big = pc2.tile([128, 8, 128], fp32, tag="pc2", name="psC2")
                # a matmul's start=True wipes its ENTIRE psum bank, so the
                # 512B-strided piece windows can't use start bits: zero the
                # accumulation region once on DVE and accumulate-only.
                nc.vector.memset(big[:, :, 0:DK + 1], 0.0)

                def scores(jk, off):
                    kt_sl = kt_h[:, 128 * jk:128 * (jk + 1)][:, None, :] \
                        .to_broadcast((128, 2, 128))
                    psL = pl.tile([128, 1024], fp32, tag="pl", name="psL")
                    for lo, hi in ((off, 512), (max(off, 512), 1024)):
                        if lo >= hi:
                            continue
                        nc.tensor.matmul(
                            psL[:, lo:hi], kt_sl,
                            qt_h[:, :, 1024 * pr + lo:1024 * pr + hi],
                            start=True, stop=True, perf_mode=DR)
                    at = apool.tile([128, 1024], bf16, tag="at", name="at")
                    nc.scalar.activation(at[:, off:1024], psL[:, off:1024],
                                         Exp, scale=0.125)
                    if 1024 * pr <= 128 * jk < 1024 * (pr + 1):
                        nc.gpsimd.tensor_mul(at[:, off:off + 128],
                                             at[:, off:off + 128], mask_sb[:])
                    if DEBUG_DUMPS and pr == 0 and h == 0 and jk == 0:
                        nc.sync.dma_start(dbg["at00"].ap(), at[:])
                    return at

                def av(jk, off, at):
                    vt_sl = vt[jk][:, (DK + 1) * h:(DK + 1) * (h + 1)]
                    for i in range(off // 128, 8):
                        nc.tensor.matmul(
                            big[:, i, 0:DK + 1],
                            at[:, 128 * i:128 * (i + 1)], vt_sl,
                            start=False, stop=(jk == i + 8 * pr),
                            skip_group_check=True)

                offs = [max(0, 128 * jk - 1024 * pr) for jk in range(nK)]
                if DEBUG_DUMPS and pr == 0 and h == 0:
                    nc.sync.dma_start(dbg["kt0"].ap(), kt8[0][:])
                    nc.sync.dma_start(dbg["qt0"].ap(), qt8[0][:])
                    nc.sync.dma_start(dbg["vt0"].ap(), vt[0][:])
                prev = None
                for jk in range(nK + 1):
                    if jk < nK:
                        at = scores(jk, offs[jk])
                    if prev is not None:
                        # yield BEFORE av: run_interleaved's filler lands
                        # between scores(jk) and av(jk-1) in the in-order PE
                        # queue, i.e. exactly where av would wait on exp.
                        yield
                        av(prev, offs[prev], prev_at)
                    prev, prev_at = (jk, at) if jk < nK else (None, None)
                # normalize: per-partition denominators at big[:, :, 64].
                # The pair's two heads interleave their normalized context
                # into one [q, i, (head, d)] stage; a single XBAR DMA
                # transpose per (pr, pair) then lands the exact ct layout
                # [head*64+d, i, q] -- no PE transposes, no extra psum.
                m, hp = h // 2, h % 2
                if DEBUG_DUMPS and pr == 0 and h == 0:
                    bigstg = cnpool.tile([128, 8, 128], fp32, tag="bigstg",
                                         name="bigstg")
                    nc.vector.tensor_copy(bigstg[:], big[:])
                    nc.sync.dma_start(dbg["big00"].ap(), bigstg[:])
                rc = rcpool.tile([128, 8, 1], fp32, tag="rc", name="rc")
                nc.vector.reciprocal(rc[:], big[:, :, DK:DK + 1])
                key = (pr, m)
                if key not in pair_cn:
                    pair_cn[key] = cnpool.tile([128, 8, 2, DK], bf16,
                                               tag="cn", name="cn")
                cn = pair_cn[key]
                nc.vector.tensor_mul(cn[:, :, hp, :], big[:, :, 0:DK],
                                     rc[:].to_broadcast((128, 8, DK)))
                if hp == 1:
                    yield
                    # two half-transposes: the first 512 ct columns land
                    # ~1.5us earlier, unblocking consumers of the window
                    for hw_ in range(4):
                        cs = ct[m][:, 1024 * pr + 256 * hw_:
                                   1024 * pr + 256 * (hw_ + 1)]
                        nc.sync.dma_start_transpose(
                            cs.rearrange("p (i q) -> p i q", q=128),
                            pair_cn[key][:, 2 * hw_:2 * (hw_ + 1)]
                            .rearrange("p a b c -> p (a b c)"))
                    if DEBUG_DUMPS and pr == 0 and m == 0:
                        nc.sync.dma_start(dbg["cn0"].ap(), pair_cn[key][:])
                    pair_cn.pop(key)

            def oproj_steps(pr, i0, i1):
                # pr=1 runs after attention: the psL pool (pl) is dead then,
                # so borrow its [128,1024] tiles as psO slot pairs to deepen
                # the psum rotation (2 pv + 4 pl-halves = 6 slots) and hide
                # the DVE-copy + semaphore latency per unit.
                slots = []
                if pr == 1:
                    for _ in range(2):
                        big = pl.tile([128, 1024], fp32, tag="pl", name="psOb")
                        slots += [big[:, 0:512], big[:, 512:1024]]
                    # pc2's 2 banks are dead after attention: 2 more slots
                    bigc = pc2.tile([128, 8, 128], fp32, tag="pc2",
                                    name="psOc")
                    slots += [
                        bigc[:, 4 * i:4 * (i + 1), :]
                        .rearrange("p a b -> p (a b)") for i in range(2)]
                cnt = 0
                nslot = len(slots)
                for i in range(8 * pr + i0, 8 * pr + i1):
                    # one [128, D] output row-chunk per i: two psum halves
                    # drained into one osb stage, one store DMA; tail
                    # stores ride the Pool SWDGE queue (no HWDGE).
                    osb = opool.tile([128, D], bf16, tag="osb", name="osb")
                    last = (pr == 1 and i == 8 * pr + i1 - 1)
                    for n in range(D // 512):
                        if pr == 1 and cnt % 4 != 0:
                            psO = slots[(cnt - cnt // 4 - 1) % nslot]
                        else:
                            psO = pv.tile([128, 512], fp32, tag="pv",
                                          name="psO")
                        cnt += 1
                        for m in range(4):
                            nc.tensor.matmul(
                                psO[:], ct[m][:, 128 * i:128 * (i + 1)],
                                wo_sb[:, m, 512 * n:512 * (n + 1)],
                                start=(m == 0), stop=(m == 3))
                        nc.vector.tensor_copy(
                            osb[:, 512 * n:512 * (n + 1)], psO[:])
                        if last:
                            # final chunk: store each half the moment its
                            # psum drains, on sync (no Pool SWDGE prep in
                            # the kernel's tail latency chain)
                            nc.sync.dma_start(
                                out.ap()[128 * i:128 * (i + 1),
                                         512 * n:512 * (n + 1)],
                                osb[:, 512 * n:512 * (n + 1)])
                        if n == 0:
                            yield
                    if not last:
                        eng = nc.gpsimd if pr == 1 else nc.sync
                        eng.dma_start(out.ap()[128 * i:128 * (i + 1), :],
                                      osb[:])
                    yield

            # ---- prologue: K-m0, Q-m0 first halves only -----------------
            # The attention phases are ScalarE(exp)-bound, so as much
            # projection PE work as possible is deferred INTO them as
            # fillers. Attention units are emitted BEFORE their filler
            # slab: the Tile scheduler picks ready work in priority
            # (emission) order, so exp-feeding attention matmuls win PE
            # when ready and projections fill the gaps. Only kt/qt[0:1024]
            # is needed by the pr0 units, so the first group starts right
            # after K0/Q0's js 0-1; js 2-3 run as its earliest fillers.
            # wo is only needed at the output projection (mid-kernel):
            # load it AFTER the attention-critical x chunks on the
            # in-order SP queue.
            nc.sync.dma_start(wo_sb[:],
                               wo.ap().rearrange("(m p) d -> p m d", p=128))
            # K0/Q0 first halves are emitted BEFORE the V blocks: they are
            # the critical path to the first exp, and the priority scheduler
            # runs V's matmuls whenever their inputs arrive anyway.
            for jb in range(0, SKC, VB):
                vproj_block(jb)
            for js in range(2):
                kproj_js(0, js)
            for js in range(2):
                qproj_js(0, js)

            def run_interleaved(attn_gens, filler_steps, every,
                                overlap=None):
                """Step attention units in order, but start the next unit's
                generator `overlap` steps before the current one ends: its
                scores+exp emission overlaps the current unit's drain, so
                ScalarE never idles across unit boundaries (the new unit's
                first AV naturally waits for the psC pool)."""
                if overlap is None:
                    overlap = TUNE.get("overlap", 0)
                n = 0
                fillers = iter(filler_steps)
                units = [[g, ns] for g, ns in attn_gens]
                active = []
                i = 0
                while i < len(units) or active:
                    if not active and i < len(units):
                        active.append(units[i]); i += 1
                    if (len(active) == 1 and active[0][1] <= overlap
                            and i < len(units)):
                        active.append(units[i]); i += 1
                    for a in list(active):
                        if a[1] <= 0:
                            # deferred exhaust: the unit's final AV +
                            # normalize emit AFTER the next unit's first
                            # scores (emitted last pass), so ScalarE chews
                            # the new unit's exps during this unit's drain.
                            for _ in a[0]:
                                pass
                            active.remove(a)
                            continue
                        try:
                            next(a[0])
                            a[1] -= 1
                        except StopIteration:
                            active.remove(a)
                            continue
                        n += 1
                        if n % every == 0:
                            f = next(fillers, None)
                            if f is not None:
                                f()
                for f in fillers:
                    f()

            # Mixed groups: each group runs a pr0 head-pair TOGETHER with
            # earlier pairs' pr1 units (pr1 only needs its own kt/qt, never
            # pr0), so every filler slab sits inside an exp-rich span and
            # ScalarE stays the binding resource throughout. Head-pair m's
            # K/Q projections fill group m-1 (done before pair m's pr0 unit
            # starts); K0/Q0's second halves fill group 0 ahead of pr1(0).
            # The last group's oproj(0) fillers need ALL pr0 units done, so
            # pr0(6,7) fold into group 3 and pr1(6,7) run last.
            def kq_fill(m):
                return ([lambda js=js: kproj_js(m, js) for js in range(SQT)]
                        + [lambda js=js: qproj_js(m, js) for js in range(SQT)])

            g1_fill = ([lambda js=js: kproj_js(0, js) for js in (2, 3)]
                       + [lambda js=js: qproj_js(0, js) for js in (2, 3)]
                       + kq_fill(1))
            groups = TUNE.get("groups", (
                ([(0, 0), (0, 1), (1, 0), (1, 1)], "g1", 3),
                ([(0, 2), (0, 3), (1, 2), (1, 3)], 2, 4),
                ([(0, 4), (0, 5), (1, 4), (1, 5), (0, 6), (0, 7)], 3, 3),
            ))
            for units, fm, ev in groups:
                fill = g1_fill if fm == "g1" else (
                    kq_fill(fm) if fm is not None else [])
                run_interleaved(
                    [(attn_steps(pr, h), 8 * (pr + 1)) for pr, h in units],
                    fill, every=ev)

            # last pr1 pair with oproj(0) dribbled in
            ofill0 = oproj_steps(0, 0, 8)
            run_interleaved(
                [(attn_steps(1, 6), 16), (attn_steps(1, 7), 16)],
                [lambda: next(ofill0, None) for _ in range(16)],
                every=TUNE.get("every5", 2))
            for _ in oproj_steps(1, 0, 8):
                pass
            if DEBUG_DUMPS:
                nc.sync.dma_start(dbg["ct0"].ap(), ct[0][:])
    return nc


def make_masks():
    """Diagonal triangle mask [sk_r, sq_c]: keep (1.0) where c >= r."""
    r = np.arange(128)[:, None]
    c = np.arange(128)[None, :]
    return (c >= r).astype(BF16)


def make_in_maps(queries, keys, values, Wq, Wk, Wv, Wo):
    masks = make_masks()
    # per-batch transposed bf16 activations, shared by both head-group cores
    xT = {}
    for b in range(B):
        xT[b] = tuple(
            np.ascontiguousarray(np.asarray(x)[b].astype(BF16).T)
            for x in (queries, keys, values))
    wg = {}
    for g in range(2):
        sl = slice(g * GW, (g + 1) * GW)
        wg[g] = (np.asarray(Wq)[:, sl].astype(BF16),
                 np.asarray(Wk)[:, sl].astype(BF16),
                 np.asarray(Wv)[:, sl].astype(BF16),
                 np.ascontiguousarray(np.asarray(Wo)[sl, :]).astype(BF16))
    in_maps = []
    for c in range(N_CORES):
        b, g = c // 2, c % 2
        q, k, v = xT[b]
        wq_, wk_, wv_, wo_ = wg[g]
        in_maps.append({"xqT": q, "xkT": k, "xvT": v, "wq": wq_, "wk": wk_,
                        "wv": wv_, "wo": wo_, "masks": masks})
    return in_maps


def kernel(queries, keys, values, mask, Wq, Wk, Wv, Wo, bo):
    _apply_compat_patches()
    from concourse.bass_utils import run_bass_kernel_spmd

    key = "attn"
    if key not in _nc_cache:
        _nc_cache[key] = build_attention_nc()
    nc = _nc_cache[key]

    in_maps = make_in_maps(queries, keys, values, Wq, Wk, Wv, Wo)
    res = run_bass_kernel_spmd(nc, in_maps, core_ids=list(range(N_CORES)))

    out = np.empty((B, S, D), dtype=np.float32)
    for b in range(B):
        out[b] = (res.results[2 * b]["out"].astype(np.float32)
                  + res.results[2 * b + 1]["out"].astype(np.float32))
    out += bo.astype(np.float32)[None, None, :]
    return out

